# revision 41
# baseline (speedup 1.0000x reference)
"""Trainium2 Bass kernel: multi-head attention (dense transformer block).

Computation (per batch b):
    Q = x @ Wq + bq ; K = x @ Wk + bk ; V = x @ Wv + bv        (per head)
    P = exp((Q @ K^T) / sqrt(Dh))            (no max-subtraction: scores O(1))
    out = sum_h (P @ V / rowsum(P)) @ Wd[h] + bd

Sharding (data + tensor parallel): 8 cores; core c handles batch b = c // 4
and the 4 heads starting at 4*(c % 4). Each core computes a partial [L, D]
output; the host sums the 4 partials per batch and adds bd. Host-side input
marshalling (layout only, no FLOPs): x is passed pre-transposed as bf16
x^T [DMODEL, L] per core and the weight slices as bf16, so the device
spends no time on x transposes or fp32->bf16 weight casts.

Schedule: the span is bounded by PE matmul streaming (~175us) with the
Scalar(ACT) exp stream (~125us) hidden under it:
  - 4 "phases", one per (pair, exp-chunk): scores + exp, paced by PSUM
    recycling. Between score l-tiles we pump "filler" PE work that is
    already data-ready: V^T projection (phase A), the previous phase's
    attend chunks, Q ec1 projection, out-projection + y DMA (phases B-D).
  - scores run as two concurrent 64-row PE tiles (tile_position row groups
    0/64), interleaved h0/h1 so the second tile's LDWEIGHTS pulls ahead.
  - V is produced as V^T (weights stationary, N=512 streams) and moved into
    [l', d] attend layout by DMA-transpose (2-byte xbar path) - zero PE cost.
  - softmax denominator via 64 replicated ones-columns in the attend
    stationary operand (free: matmul time ~ N only).
  - normalize: DVE copy (frees PSUM), DVE reciprocal_approx_fast, multiply
    on the otherwise-idle GpSimd.
  - ACT does exp ONLY while busy (weight DMAs are issued from its idle DGE
    queue at startup; tail y drains use it after the last exp).
All matmuls bf16 (fp32 PSUM accumulation); rel err vs fp32 ref ~5e-3.
"""

import os
import sys
from contextlib import ExitStack

import numpy as np
import ml_dtypes

for _p in ("/opt/trn_rl_repo", "/root/.axon_site/_ro/trn_rl_repo"):
    if os.path.isdir(_p) and _p not in sys.path:
        sys.path.append(_p)

import concourse.bass as bass
import concourse.tile as tile
from concourse import bacc, mybir
from concourse.bass import ds
from concourse.bass_utils import run_bass_kernel_spmd

F32 = mybir.dt.float32
BF16 = mybir.dt.bfloat16
BF16_NP = ml_dtypes.bfloat16

# Problem sizes (hardcoded per contract).
DMODEL, HEADS, DHEAD = 1024, 16, 64
B, L = 2, 2048
NCORES = 8
H_PER_CORE = B * HEADS // NCORES          # 4 heads per core
NPAIR = H_PER_CORE // 2                   # head pairs per core
P = 128                                   # partitions
KT = DMODEL // P                          # 8 k-tiles over dmodel
NLT = L // P                              # 16 l-tiles
LCH = 512                                 # matmul free-dim chunk (one psum bank)
ECH = 1024                                # exp chunk (2 psum banks)
NEC = L // ECH                            # 2 exp chunks
MCH = 512                                 # m-chunk for out-proj
NMC = DMODEL // MCH
NSUB = ECH // LCH                         # 2 sub-chunks per exp chunk


def build_nc():
    """Build the SPMD Bass program for one core."""
    nc = bacc.Bacc("TRN2", target_bir_lowering=False, debug=False,
                   num_devices=NCORES)

    xt_d = nc.dram_tensor("xt", [DMODEL, L], BF16, kind="ExternalInput").ap()
    wq_d = nc.dram_tensor("wq", [DMODEL, H_PER_CORE * DHEAD], BF16, kind="ExternalInput").ap()
    wk_d = nc.dram_tensor("wk", [DMODEL, H_PER_CORE * DHEAD], BF16, kind="ExternalInput").ap()
    wv_d = nc.dram_tensor("wv", [DMODEL, H_PER_CORE * DHEAD], BF16, kind="ExternalInput").ap()
    wd_d = nc.dram_tensor("wd", [H_PER_CORE * DHEAD, DMODEL], BF16, kind="ExternalInput").ap()
    bq_d = nc.dram_tensor("bq", [H_PER_CORE * DHEAD], F32, kind="ExternalInput").ap()
    bk_d = nc.dram_tensor("bk", [H_PER_CORE * DHEAD], F32, kind="ExternalInput").ap()
    bv_d = nc.dram_tensor("bv", [H_PER_CORE * DHEAD], F32, kind="ExternalInput").ap()
    y_d = nc.dram_tensor("y", [L, DMODEL], F32, kind="ExternalOutput").ap()
    dbg = {}
    if os.environ.get("K_DEBUG"):
        for nm, shape in (("kT0", [P, L]), ("qT0", [P, L]), ("vT0", [P, L]),
                          ("vt0", [P, NLT, 2, P]), ("on0", [P, NPAIR, L]),
                          ("pt0", [P, ECH]), ("pt15", [P, ECH])):
            dbg[nm] = nc.dram_tensor(nm, shape, BF16, kind="ExternalOutput").ap()

    with ExitStack() as ctx:
        tc = ctx.enter_context(tile.TileContext(nc))
        _body(nc, tc, ctx, xt_d, wq_d, wk_d, wv_d, wd_d, bq_d, bk_d, bv_d, y_d,
              dbg)
    nc.compile()
    return nc


def _body(nc, tc, ctx, xt_d, wq_d, wk_d, wv_d, wd_d, bq_d, bk_d, bv_d, y_d,
          dbg=None):
    const = ctx.enter_context(tc.tile_pool(name="const", bufs=1))
    sb = ctx.enter_context(tc.tile_pool(name="sb", bufs=1))
    psum = ctx.enter_context(tc.tile_pool(name="psum", bufs=1, space="PSUM"))

    # biases via gpsimd SWDGE (off the hw queues)
    bias_sb = const.tile([P, 3, NPAIR], F32)
    for i, b_d in enumerate((bq_d, bk_d, bv_d)):
        for p in range(NPAIR):
            nc.gpsimd.dma_start(bias_sb[:, i, p:p + 1],
                                b_d.rearrange("(a p) -> a p", p=P)[p:p + 1, :]
                                .rearrange("a p -> p a"))

    # ---- weights: bf16 from host, DMA'd on the scalar queue (idle now);
    # ordered so the K projections (emitted first) unblock earliest ----
    w_sb = const.tile([P, NPAIR, 3, KT, P], BF16)
    wd_sb = const.tile([P, NPAIR, DMODEL], BF16)
    for i, p in ((1, 0), (0, 0), (1, 1), (0, 1), (2, 0), (2, 1)):
        w_d = (wq_d, wk_d, wv_d)[i]
        nc.scalar.dma_start(
            w_sb[:, p, i],
            w_d.rearrange("(kt k) m -> k kt m", k=P)[:, :, ds(p * P, P)])
    nc.scalar.dma_start(wd_sb, wd_d.rearrange("(pp k) m -> k pp m", k=P))

    # ---- x^T: bf16 from host, 4 L-chunk DMAs so K-proj starts early ----
    xt = sb.tile([P, KT, L], BF16)
    for lc in range(4):
        for kh in range(2):
            nc.sync.dma_start(
                xt[:, ds(4 * kh, 4), ds(lc * LCH, LCH)],
                xt_d.rearrange("(kt p) l -> p kt l", p=P)
                [:, ds(4 * kh, 4), ds(lc * LCH, LCH)])

    # ---- persistent activations ----
    kT = [sb.tile([P, L], BF16, tag=f"kT{p}", bufs=1, name=f"kT{p}")
          for p in range(NPAIR)]
    qT = [sb.tile([P, L], BF16, tag=f"qT{p}", bufs=1, name=f"qT{p}")
          for p in range(NPAIR)]
    vTs = [sb.tile([P, L], BF16, tag=f"vT{p}", bufs=1, name=f"vT{p}")
           for p in range(NPAIR)]
    vt = [sb.tile([P, NLT, 2 * P], BF16, tag=f"vt{p}", bufs=1, name=f"vt{p}")
          for p in range(NPAIR)]
    o_norm = sb.tile([P, NPAIR, L], BF16)

    # pt tiles: exp outputs; ring holds the previous phase's 32 tiles plus
    # the current phase's in-flight ones.
    PT_BUFS = 42

    def pt_tile():
        return sb.tile([P, ECH], BF16, tag="pt", bufs=PT_BUFS, name="pt")

    # psum: scores 2x[128,1024] (4 banks) + attend op 2x[128,512] (2 banks)
    # + aux (projections/out-proj) 2x[128,512] (2 banks)
    def sc_tile():
        return psum.tile([P, ECH], F32, tag="sctr", bufs=2, name="sctr")

    def aux_tile(shape, dt=F32, name="aux"):
        return psum.tile(shape, dt, tag="aux", bufs=2, name=name)

    # ---- helper: one qkv projection 512-chunk (8 accumulating matmuls) ----
    def qkv_chunk(dst, p, i, lc):
        ps = aux_tile([P, LCH], name="qkvp")
        for kt in range(KT):
            nc.tensor.matmul(
                ps, lhsT=w_sb[:, p, i, kt],
                rhs=xt[:, kt, ds(lc * LCH, LCH)],
                start=(kt == 0), stop=(kt == KT - 1))
        nc.vector.tensor_scalar_add(
            dst[:, ds(lc * LCH, LCH)], ps, bias_sb[:, i, p:p + 1])

    # ---- filler units ----
    def vT_chunk_unit(p, lc):
        def emit():
            qkv_chunk(vTs[p], p, 2, lc)
        return emit

    def v_trans_unit(p, lts):
        """One DMA-transpose per l-tile into vt layout
        [1s(0:64) | V_h0(64:128) | V_h1(128:192) | 1s(192:256)], so
        h0's attend lhsT = cols 0:128 = [ones | V] and h1's = 128:256 =
        [V | ones] - both contiguous."""
        def emit():
            for lt in lts:
                nc.sync.dma_start_transpose(
                    vt[p][:, lt, ds(DHEAD, 2 * DHEAD)],
                    vTs[p][:, ds(lt * P, P)])
        return emit

    def ones_unit(p):
        def emit():
            nc.vector.memset(vt[p][:, :, 0:DHEAD], 1.0)
            nc.vector.memset(vt[p][:, :, 3 * DHEAD:4 * DHEAD], 1.0)
        return emit

    def q1_unit(p, lc):
        def emit():
            qkv_chunk(qT[p], p, 0, lc)
        return emit

    def attend_unit(pt_tiles, p, ec, sub, h):
        """One attend chunk: 16 accumulating matmuls + normalize chain."""
        def emit():
            lc = ec * ECH + sub * LCH
            op = psum.tile([P, LCH], F32, tag="op", bufs=2, name="op")
            for lt in range(NLT):
                nc.tensor.matmul(
                    op, lhsT=vt[p][:, lt, ds(P * h, P)],
                    rhs=pt_tiles[h][lt][:, ds(sub * LCH, LCH)],
                    start=(lt == 0), stop=(lt == NLT - 1))
            # h0: rows 0:64 = denominator, 64:128 = O; h1: the reverse.
            # Both halves are copied to base-partition-0 tiles (custom DVE
            # ops and TensorTensor need base-0-aligned SBUF operands)
            o_rows = ds(DHEAD, DHEAD) if h == 0 else ds(0, DHEAD)
            d_rows = ds(0, DHEAD) if h == 0 else ds(DHEAD, DHEAD)
            os_sb = sb.tile([DHEAD, LCH], F32, tag="os", bufs=2, name="os_sb")
            nc.vector.tensor_copy(os_sb, op[o_rows, :])
            den = sb.tile([DHEAD, LCH], F32, tag="den", bufs=2, name="den")
            nc.vector.tensor_copy(den, op[d_rows, :])
            rs = sb.tile([DHEAD, LCH], F32, tag="rs", bufs=2, name="rs")
            nc.vector.reciprocal_approx_fast(rs, den)
            nc.gpsimd.tensor_mul(
                o_norm[ds(64 * h, 64), p, ds(lc, LCH)],
                os_sb, rs)
        return emit

    def attend_split(pt_tiles, p, ec, sub, h, cut):
        """Attend chunk split at l-tile `cut`: part A (lts < cut) can run
        as soon as those exps land; part B finishes the accumulation and
        normalizes. The op PSUM slot stays held between the parts."""
        state = {}

        def emit_a():
            op = psum.tile([P, LCH], F32, tag="op", bufs=2, name="op")
            state["op"] = op
            for lt in range(cut):
                nc.tensor.matmul(
                    op, lhsT=vt[p][:, lt, ds(P * h, P)],
                    rhs=pt_tiles[h][lt][:, ds(sub * LCH, LCH)],
                    start=(lt == 0), stop=False)

        def emit_b():
            lc = ec * ECH + sub * LCH
            op = state["op"]
            for lt in range(cut, NLT):
                nc.tensor.matmul(
                    op, lhsT=vt[p][:, lt, ds(P * h, P)],
                    rhs=pt_tiles[h][lt][:, ds(sub * LCH, LCH)],
                    start=False, stop=(lt == NLT - 1))
            o_rows = ds(DHEAD, DHEAD) if h == 0 else ds(0, DHEAD)
            d_rows = ds(0, DHEAD) if h == 0 else ds(DHEAD, DHEAD)
            os_sb = sb.tile([DHEAD, LCH], F32, tag="os", bufs=2, name="os_sb")
            nc.vector.tensor_copy(os_sb, op[o_rows, :])
            den = sb.tile([DHEAD, LCH], F32, tag="den", bufs=2, name="den")
            nc.vector.tensor_copy(den, op[d_rows, :])
            rs = sb.tile([DHEAD, LCH], F32, tag="rs", bufs=2, name="rs")
            nc.vector.reciprocal_approx_fast(rs, den)
            nc.gpsimd.tensor_mul(
                o_norm[ds(64 * h, 64), p, ds(lc, LCH)], os_sb, rs)

        return emit_a, emit_b

    def outproj_unit(lt, tail=False):
        """Y[lt-tile, :] = sum_pairs o_norm^T @ Wd, drain + DMA out."""
        def emit():
            for mc in range(NMC):
                yp = aux_tile([P, MCH], name="yp")
                for p in range(NPAIR):
                    nc.tensor.matmul(
                        yp, lhsT=o_norm[:, p, ds(lt * P, P)],
                        rhs=wd_sb[:, p, ds(mc * MCH, MCH)],
                        start=(p == 0), stop=(p == NPAIR - 1))
                ys = sb.tile([P, MCH], F32, tag="ys", bufs=3, name="ys")
                if tail:
                    nc.scalar.copy(ys, yp)   # ACT is idle after the last exp
                else:
                    nc.vector.tensor_copy(ys, yp)
                nc.sync.dma_start(y_d[ds(lt * P, P), ds(mc * MCH, MCH)], ys)
        return emit

    # ---- startup: the minimum for phase A's first score tiles (K g0 +
    # Q ec0); everything else becomes phase-A fillers so the first exp
    # fires as early as the DMAs allow ----
    qkv_chunk(kT[0], 0, 1, 0)
    for lc in range(NSUB):
        qkv_chunk(qT[0], 0, 0, lc)

    # ---- phases: scores+exp paced by ACT; fillers keep PE dense ----
    def phase(p, ec, fillers, h_major=False, fillers2=None, out_pt=None):
        """Scores+exp for (pair p, exp chunk ec); pump filler units between
        l-tiles, front-loaded (done ~3 l-tiles early) so the PE reaches the
        next phase's scores before this phase's last exps finish. With
        h_major, the two heads run as separate sweeps (fillers2 pumps during
        the h1 sweep - it may reference this phase's own h0 pt tiles).
        Returns the pt tiles for this phase."""
        pt_tiles = out_pt if out_pt is not None else [[None] * NLT, [None] * NLT]

        def pump(fillers, fi, lt):
            nf = len(fillers)
            while fi < nf and fi * (NLT - 1) < nf * (lt + 1):
                fillers[fi]()
                fi += 1
            return fi

        if not h_major:
            fi = 0
            for lt in range(NLT):
                sp = [sc_tile(), sc_tile()]
                for sub in range(NSUB):   # h-interleaved: dual 64-row tiles
                    for h in range(2):
                        nc.tensor.matmul(
                            sp[h][:, ds(sub * LCH, LCH)],
                            lhsT=kT[p][ds(64 * h, 64), ds(lt * P, P)],
                            rhs=qT[p][ds(64 * h, 64),
                                      ds(ec * ECH + sub * LCH, LCH)],
                            start=True, stop=True)
                for h in range(2):
                    pt = pt_tile()
                    nc.scalar.activation(
                        pt, sp[h], func=mybir.ActivationFunctionType.Exp,
                        scale=1.0 / np.sqrt(DHEAD))
                    pt_tiles[h][lt] = pt
                fi = pump(fillers, fi, lt)
        else:
            for h in range(2):
                fl = fillers if h == 0 else (fillers2 or [])
                fi = 0
                for lt in range(NLT):
                    sp = sc_tile()
                    for sub in range(NSUB):
                        nc.tensor.matmul(
                            sp[:, ds(sub * LCH, LCH)],
                            lhsT=kT[p][ds(64 * h, 64), ds(lt * P, P)],
                            rhs=qT[p][ds(64 * h, 64),
                                      ds(ec * ECH + sub * LCH, LCH)],
                            start=True, stop=True)
                    pt = pt_tile()
                    nc.scalar.activation(
                        pt, sp, func=mybir.ActivationFunctionType.Exp,
                        scale=1.0 / np.sqrt(DHEAD))
                    pt_tiles[h][lt] = pt
                    fi = pump(fl, fi, lt)
        return pt_tiles

    # phase A: (p0, e0); fillers: p1's K/Q-ec0 first (phase B needs them),
    # then V^T proj + V DMA-transposes for both pairs, Q ec1 for p0
    fillA = [lambda: qkv_chunk(kT[0], 0, 1, 1),
             lambda: qkv_chunk(kT[0], 0, 1, 2),
             lambda: qkv_chunk(kT[0], 0, 1, 3),
             lambda: qkv_chunk(kT[1], 1, 1, 0),
             lambda: qkv_chunk(kT[1], 1, 1, 1),
             lambda: qkv_chunk(kT[1], 1, 1, 2),
             lambda: qkv_chunk(kT[1], 1, 1, 3),
             lambda: qkv_chunk(qT[1], 1, 0, 0),
             lambda: qkv_chunk(qT[1], 1, 0, 1),
             ones_unit(0), ones_unit(1)]
    for p in range(NPAIR):
        for g in range(4):
            fillA.append(vT_chunk_unit(p, g))
            fillA.append(v_trans_unit(p, range(4 * g, 4 * g + 4)))
    fillA += [q1_unit(0, NSUB + 0), q1_unit(0, NSUB + 1)]
    ptA = phase(0, 0, fillA)

    # phase B: (p1, e0); fillers: attend(p0,e0) h-major (frees pt(A) fast)
    fillB = [attend_unit(ptA, 0, 0, 0, 0),
             attend_unit(ptA, 0, 0, 1, 0),
             attend_unit(ptA, 0, 0, 0, 1),
             attend_unit(ptA, 0, 0, 1, 1)]
    ptB = phase(1, 0, fillB)

    # phase C: (p0, e1); fillers: attend(p1,e0), Q ec1 for p1, then
    # out-proj rows 0:512
    fillC = [attend_unit(ptB, 1, 0, 0, 0),
             attend_unit(ptB, 1, 0, 1, 0),
             attend_unit(ptB, 1, 0, 0, 1),
             attend_unit(ptB, 1, 0, 1, 1),
             q1_unit(1, NSUB + 0), q1_unit(1, NSUB + 1)]
    fillC += [outproj_unit(lt) for lt in range(4)]
    ptC = phase(0, 1, fillC)

    # phase D: (p1, e1); fillers: attend(p0,e1) + out-proj rows 512:1024
    fillD = [attend_unit(ptC, 0, 1, 0, 0),
             attend_unit(ptC, 0, 1, 1, 0),
             attend_unit(ptC, 0, 1, 0, 1),
             attend_unit(ptC, 0, 1, 1, 1)]
    fillD += [outproj_unit(lt) for lt in range(4, 8)]
    ptD = phase(1, 1, fillD)

    # tail: attend(p1,e1) + out-proj rows 1024:2048. The h0 chunks are
    # split at lt=8 so their first halves stream while the last h0/h1 exps
    # are still on the ACT engine.
    a00, b00 = attend_split(ptD, 1, 1, 0, 0, 8)
    a10, b10 = attend_split(ptD, 1, 1, 1, 0, 8)
    a00(); a10(); b00(); b10()
    attend_unit(ptD, 1, 1, 0, 1)()
    for lt in range(8, 12):
        outproj_unit(lt, tail=True)()
    attend_unit(ptD, 1, 1, 1, 1)()
    for lt in range(12, 16):
        outproj_unit(lt, tail=True)()

    if dbg:
        nc.sync.dma_start(dbg["kT0"], kT[0])
        nc.sync.dma_start(dbg["qT0"], qT[0])
        nc.sync.dma_start(dbg["vT0"], vTs[0])
        nc.sync.dma_start(dbg["vt0"], vt[0])
        nc.sync.dma_start(dbg["on0"], o_norm)
        nc.sync.dma_start(dbg["pt0"], ptD[0][0])
        nc.sync.dma_start(dbg["pt15"], ptD[1][15])


_NC_CACHE = {}


def _get_nc():
    if "nc" not in _NC_CACHE:
        _NC_CACHE["nc"] = build_nc()
    return _NC_CACHE["nc"]


def shard_inputs(x, Wq, bq, Wk, bk, Wv, bv, Wd, bd):
    """Build the 8 per-core input maps (layout marshalling only)."""
    in_maps = []
    x = np.asarray(x, np.float32)
    for c in range(NCORES):
        b = c // (NCORES // B)
        h0 = (c % (NCORES // B)) * H_PER_CORE
        hs = slice(h0, h0 + H_PER_CORE)
        in_maps.append({
            "xt": np.ascontiguousarray(x[b].T).astype(BF16_NP),
            "wq": np.ascontiguousarray(np.asarray(Wq[:, hs, :]).reshape(DMODEL, -1)).astype(BF16_NP),
            "wk": np.ascontiguousarray(np.asarray(Wk[:, hs, :]).reshape(DMODEL, -1)).astype(BF16_NP),
            "wv": np.ascontiguousarray(np.asarray(Wv[:, hs, :]).reshape(DMODEL, -1)).astype(BF16_NP),
            "wd": np.ascontiguousarray(np.asarray(Wd[hs]).reshape(-1, DMODEL)).astype(BF16_NP),
            "bq": np.ascontiguousarray(np.asarray(bq[hs], np.float32).reshape(-1)),
            "bk": np.ascontiguousarray(np.asarray(bk[hs], np.float32).reshape(-1)),
            "bv": np.ascontiguousarray(np.asarray(bv[hs], np.float32).reshape(-1)),
        })
    return in_maps


def gather_outputs(results, bd):
    """Sum partial outputs per batch and add bd."""
    out = np.zeros((B, L, DMODEL), np.float32)
    per_b = NCORES // B
    for c, res in enumerate(results):
        out[c // per_b] += res["y"]
    out += np.asarray(bd, np.float32)[None, None, :]
    return out


def kernel(x, Wq, bq, Wk, bk, Wv, bv, Wd, bd, _trace=False):
    nc = _get_nc()
    in_maps = shard_inputs(x, Wq, bq, Wk, bk, Wv, bv, Wd, bd)
    res = run_bass_kernel_spmd(nc, in_maps, list(range(NCORES)), trace=_trace)
    out = gather_outputs(res.results, bd)
    if _trace:
        kernel.last_results = res
    return out


# revision 42
# speedup vs baseline: 1.0163x; 1.0163x over previous
"""Trainium2 Bass kernel: multi-head attention (dense transformer block).

Computation (per batch b):
    Q = x @ Wq + bq ; K = x @ Wk + bk ; V = x @ Wv + bv        (per head)
    P = exp((Q @ K^T) / sqrt(Dh))            (no max-subtraction: scores O(1))
    out = sum_h (P @ V / rowsum(P)) @ Wd[h] + bd

Sharding (data + tensor parallel): 8 cores; core c handles batch b = c // 4
and the 4 heads starting at 4*(c % 4). Each core computes a partial [L, D]
output; the host sums the 4 partials per batch and adds bd. Host-side input
marshalling (layout only, no FLOPs): x is passed pre-transposed as bf16
x^T [DMODEL, L] per core and the weight slices as bf16, so the device
spends no time on x transposes or fp32->bf16 weight casts.

Schedule: the span is bounded by PE matmul streaming (~175us) with the
Scalar(ACT) exp stream (~125us) hidden under it:
  - 4 "phases", one per (pair, exp-chunk): scores + exp, paced by PSUM
    recycling. Between score l-tiles we pump "filler" PE work that is
    already data-ready: V^T projection (phase A), the previous phase's
    attend chunks, Q ec1 projection, out-projection + y DMA (phases B-D).
  - scores run as two concurrent 64-row PE tiles (tile_position row groups
    0/64), interleaved h0/h1 so the second tile's LDWEIGHTS pulls ahead.
  - V is produced as V^T (weights stationary, N=512 streams) and moved into
    [l', d] attend layout by DMA-transpose (2-byte xbar path) - zero PE cost.
  - softmax denominator via 64 replicated ones-columns in the attend
    stationary operand (free: matmul time ~ N only).
  - normalize: DVE copy (frees PSUM), DVE reciprocal_approx_fast, multiply
    on the otherwise-idle GpSimd.
  - ACT does exp ONLY while busy (weight DMAs are issued from its idle DGE
    queue at startup; tail y drains use it after the last exp).
All matmuls bf16 (fp32 PSUM accumulation); rel err vs fp32 ref ~5e-3.
"""

import os
import sys
from contextlib import ExitStack

import numpy as np
import ml_dtypes

for _p in ("/opt/trn_rl_repo", "/root/.axon_site/_ro/trn_rl_repo"):
    if os.path.isdir(_p) and _p not in sys.path:
        sys.path.append(_p)

import concourse.bass as bass
import concourse.tile as tile
from concourse import bacc, mybir
from concourse.bass import ds
from concourse.bass_utils import run_bass_kernel_spmd

F32 = mybir.dt.float32
BF16 = mybir.dt.bfloat16
BF16_NP = ml_dtypes.bfloat16

# Problem sizes (hardcoded per contract).
DMODEL, HEADS, DHEAD = 1024, 16, 64
B, L = 2, 2048
NCORES = 8
H_PER_CORE = B * HEADS // NCORES          # 4 heads per core
NPAIR = H_PER_CORE // 2                   # head pairs per core
P = 128                                   # partitions
KT = DMODEL // P                          # 8 k-tiles over dmodel
NLT = L // P                              # 16 l-tiles
LCH = 512                                 # matmul free-dim chunk (one psum bank)
ECH = 1024                                # exp chunk (2 psum banks)
NEC = L // ECH                            # 2 exp chunks
MCH = 512                                 # m-chunk for out-proj
NMC = DMODEL // MCH
NSUB = ECH // LCH                         # 2 sub-chunks per exp chunk


def build_nc():
    """Build the SPMD Bass program for one core."""
    nc = bacc.Bacc("TRN2", target_bir_lowering=False, debug=False,
                   num_devices=NCORES)

    xt_d = nc.dram_tensor("xt", [DMODEL, L], BF16, kind="ExternalInput").ap()
    wq_d = nc.dram_tensor("wq", [DMODEL, H_PER_CORE * DHEAD], BF16, kind="ExternalInput").ap()
    wk_d = nc.dram_tensor("wk", [DMODEL, H_PER_CORE * DHEAD], BF16, kind="ExternalInput").ap()
    wv_d = nc.dram_tensor("wv", [DMODEL, H_PER_CORE * DHEAD], BF16, kind="ExternalInput").ap()
    wd_d = nc.dram_tensor("wd", [H_PER_CORE * DHEAD, DMODEL], BF16, kind="ExternalInput").ap()
    bq_d = nc.dram_tensor("bq", [H_PER_CORE * DHEAD], F32, kind="ExternalInput").ap()
    bk_d = nc.dram_tensor("bk", [H_PER_CORE * DHEAD], F32, kind="ExternalInput").ap()
    bv_d = nc.dram_tensor("bv", [H_PER_CORE * DHEAD], F32, kind="ExternalInput").ap()
    y_d = nc.dram_tensor("y", [L, DMODEL], F32, kind="ExternalOutput").ap()
    dbg = {}
    if os.environ.get("K_DEBUG"):
        for nm, shape in (("kT0", [P, L]), ("qT0", [P, L]), ("vT0", [P, L]),
                          ("vt0", [P, NLT, 2, P]), ("on0", [P, NPAIR, L]),
                          ("pt0", [P, ECH]), ("pt15", [P, ECH])):
            dbg[nm] = nc.dram_tensor(nm, shape, BF16, kind="ExternalOutput").ap()

    with ExitStack() as ctx:
        tc = ctx.enter_context(tile.TileContext(nc))
        _body(nc, tc, ctx, xt_d, wq_d, wk_d, wv_d, wd_d, bq_d, bk_d, bv_d, y_d,
              dbg)
    nc.compile()
    return nc


def _body(nc, tc, ctx, xt_d, wq_d, wk_d, wv_d, wd_d, bq_d, bk_d, bv_d, y_d,
          dbg=None):
    const = ctx.enter_context(tc.tile_pool(name="const", bufs=1))
    sb = ctx.enter_context(tc.tile_pool(name="sb", bufs=1))
    psum = ctx.enter_context(tc.tile_pool(name="psum", bufs=1, space="PSUM"))

    # biases via gpsimd SWDGE (off the hw queues)
    bias_sb = const.tile([P, 3, NPAIR], F32)
    for i, b_d in enumerate((bq_d, bk_d, bv_d)):
        for p in range(NPAIR):
            nc.gpsimd.dma_start(bias_sb[:, i, p:p + 1],
                                b_d.rearrange("(a p) -> a p", p=P)[p:p + 1, :]
                                .rearrange("a p -> p a"))

    # ---- weights: bf16 from host, DMA'd on the scalar queue (idle now);
    # ordered so the K projections (emitted first) unblock earliest ----
    w_sb = const.tile([P, NPAIR, 3, KT, P], BF16)
    wd_sb = const.tile([P, NPAIR, DMODEL], BF16)
    for i, p in ((1, 0), (0, 0), (1, 1), (0, 1), (2, 0), (2, 1)):
        w_d = (wq_d, wk_d, wv_d)[i]
        nc.scalar.dma_start(
            w_sb[:, p, i],
            w_d.rearrange("(kt k) m -> k kt m", k=P)[:, :, ds(p * P, P)])
    nc.scalar.dma_start(wd_sb, wd_d.rearrange("(pp k) m -> k pp m", k=P))

    # ---- x^T: bf16 from host, 4 L-chunk DMAs so K-proj starts early ----
    xt = sb.tile([P, KT, L], BF16)
    for lc in range(4):
        for kh in range(2):
            nc.sync.dma_start(
                xt[:, ds(4 * kh, 4), ds(lc * LCH, LCH)],
                xt_d.rearrange("(kt p) l -> p kt l", p=P)
                [:, ds(4 * kh, 4), ds(lc * LCH, LCH)])

    # ---- persistent activations ----
    kT = [sb.tile([P, L], BF16, tag=f"kT{p}", bufs=1, name=f"kT{p}")
          for p in range(NPAIR)]
    qT = [sb.tile([P, L], BF16, tag=f"qT{p}", bufs=1, name=f"qT{p}")
          for p in range(NPAIR)]
    vTs = [sb.tile([P, L], BF16, tag=f"vT{p}", bufs=1, name=f"vT{p}")
           for p in range(NPAIR)]
    vt = [sb.tile([P, NLT, 2 * P], BF16, tag=f"vt{p}", bufs=1, name=f"vt{p}")
          for p in range(NPAIR)]
    o_norm = sb.tile([P, NPAIR, L], BF16)

    # pt tiles: exp outputs; ring holds the previous phase's 32 tiles plus
    # the current phase's in-flight ones.
    PT_BUFS = 42

    def pt_tile():
        return sb.tile([P, ECH], BF16, tag="pt", bufs=PT_BUFS, name="pt")

    # psum: scores 2x[128,1024] (4 banks) + attend op 2x[128,512] (2 banks)
    # + aux (projections/out-proj) 2x[128,512] (2 banks)
    def sc_tile():
        return psum.tile([P, ECH], F32, tag="sctr", bufs=2, name="sctr")

    def aux_tile(shape, dt=F32, name="aux"):
        return psum.tile(shape, dt, tag="aux", bufs=2, name=name)

    # ---- helper: one qkv projection 512-chunk (8 accumulating matmuls) ----
    def qkv_chunk(dst, p, i, lc):
        ps = aux_tile([P, LCH], name="qkvp")
        for kt in range(KT):
            nc.tensor.matmul(
                ps, lhsT=w_sb[:, p, i, kt],
                rhs=xt[:, kt, ds(lc * LCH, LCH)],
                start=(kt == 0), stop=(kt == KT - 1))
        nc.vector.tensor_scalar_add(
            dst[:, ds(lc * LCH, LCH)], ps, bias_sb[:, i, p:p + 1])

    # ---- filler units ----
    def vT_chunk_unit(p, lc):
        def emit():
            qkv_chunk(vTs[p], p, 2, lc)
        return emit

    def v_trans_unit(p, lts):
        """One DMA-transpose per l-tile into vt layout
        [1s(0:64) | V_h0(64:128) | V_h1(128:192) | 1s(192:256)], so
        h0's attend lhsT = cols 0:128 = [ones | V] and h1's = 128:256 =
        [V | ones] - both contiguous."""
        def emit():
            for lt in lts:
                nc.sync.dma_start_transpose(
                    vt[p][:, lt, ds(DHEAD, 2 * DHEAD)],
                    vTs[p][:, ds(lt * P, P)])
        return emit

    def ones_unit(p):
        def emit():
            nc.vector.memset(vt[p][:, :, 0:DHEAD], 1.0)
            nc.vector.memset(vt[p][:, :, 3 * DHEAD:4 * DHEAD], 1.0)
        return emit

    def q1_unit(p, lc):
        def emit():
            qkv_chunk(qT[p], p, 0, lc)
        return emit

    def attend_unit(pt_tiles, p, ec, sub, h):
        """One attend chunk: 16 accumulating matmuls + normalize chain."""
        def emit():
            lc = ec * ECH + sub * LCH
            op = psum.tile([P, LCH], F32, tag="op", bufs=2, name="op")
            for lt in range(NLT):
                nc.tensor.matmul(
                    op, lhsT=vt[p][:, lt, ds(P * h, P)],
                    rhs=pt_tiles[h][lt][:, ds(sub * LCH, LCH)],
                    start=(lt == 0), stop=(lt == NLT - 1))
            # h0: rows 0:64 = denominator, 64:128 = O; h1: the reverse.
            # Both halves are copied to base-partition-0 tiles (custom DVE
            # ops and TensorTensor need base-0-aligned SBUF operands)
            o_rows = ds(DHEAD, DHEAD) if h == 0 else ds(0, DHEAD)
            d_rows = ds(0, DHEAD) if h == 0 else ds(DHEAD, DHEAD)
            os_sb = sb.tile([DHEAD, LCH], F32, tag="os", bufs=2, name="os_sb")
            nc.vector.tensor_copy(os_sb, op[o_rows, :])
            den = sb.tile([DHEAD, LCH], F32, tag="den", bufs=2, name="den")
            nc.vector.tensor_copy(den, op[d_rows, :])
            rs = sb.tile([DHEAD, LCH], F32, tag="rs", bufs=2, name="rs")
            nc.vector.reciprocal_approx_fast(rs, den)
            nc.gpsimd.tensor_mul(
                o_norm[ds(64 * h, 64), p, ds(lc, LCH)],
                os_sb, rs)
        return emit

    def outproj_unit(lt, tail=False):
        """Y[lt-tile, :] = sum_pairs o_norm^T @ Wd, drain + DMA out."""
        def emit():
            for mc in range(NMC):
                yp = aux_tile([P, MCH], name="yp")
                for p in range(NPAIR):
                    nc.tensor.matmul(
                        yp, lhsT=o_norm[:, p, ds(lt * P, P)],
                        rhs=wd_sb[:, p, ds(mc * MCH, MCH)],
                        start=(p == 0), stop=(p == NPAIR - 1))
                ys = sb.tile([P, MCH], F32, tag="ys", bufs=3, name="ys")
                if tail:
                    nc.scalar.copy(ys, yp)   # ACT is idle after the last exp
                else:
                    nc.vector.tensor_copy(ys, yp)
                nc.sync.dma_start(y_d[ds(lt * P, P), ds(mc * MCH, MCH)], ys)
        return emit

    # ---- startup: the minimum for phase A's first score tiles (K g0 +
    # Q ec0); everything else becomes phase-A fillers so the first exp
    # fires as early as the DMAs allow ----
    qkv_chunk(kT[0], 0, 1, 0)
    for lc in range(NSUB):
        qkv_chunk(qT[0], 0, 0, lc)

    # ---- phases: scores+exp paced by ACT; fillers keep PE dense ----
    def phase(p, ec, fillers, h_major=False, fillers2=None, out_pt=None):
        """Scores+exp for (pair p, exp chunk ec); pump filler units between
        l-tiles, front-loaded (done ~3 l-tiles early) so the PE reaches the
        next phase's scores before this phase's last exps finish. With
        h_major, the two heads run as separate sweeps (fillers2 pumps during
        the h1 sweep - it may reference this phase's own h0 pt tiles).
        Returns the pt tiles for this phase."""
        pt_tiles = out_pt if out_pt is not None else [[None] * NLT, [None] * NLT]

        def pump(fillers, fi, lt):
            nf = len(fillers)
            while fi < nf and fi * NLT < nf * (lt + 1):
                fillers[fi]()
                fi += 1
            return fi

        if not h_major:
            fi = 0
            for lt in range(NLT):
                sp = [sc_tile(), sc_tile()]
                for sub in range(NSUB):   # h-interleaved: dual 64-row tiles
                    for h in range(2):
                        nc.tensor.matmul(
                            sp[h][:, ds(sub * LCH, LCH)],
                            lhsT=kT[p][ds(64 * h, 64), ds(lt * P, P)],
                            rhs=qT[p][ds(64 * h, 64),
                                      ds(ec * ECH + sub * LCH, LCH)],
                            start=True, stop=True)
                for h in range(2):
                    pt = pt_tile()
                    nc.scalar.activation(
                        pt, sp[h], func=mybir.ActivationFunctionType.Exp,
                        scale=1.0 / np.sqrt(DHEAD))
                    pt_tiles[h][lt] = pt
                fi = pump(fillers, fi, lt)
        else:
            for h in range(2):
                fl = fillers if h == 0 else (fillers2 or [])
                fi = 0
                for lt in range(NLT):
                    sp = sc_tile()
                    for sub in range(NSUB):
                        nc.tensor.matmul(
                            sp[:, ds(sub * LCH, LCH)],
                            lhsT=kT[p][ds(64 * h, 64), ds(lt * P, P)],
                            rhs=qT[p][ds(64 * h, 64),
                                      ds(ec * ECH + sub * LCH, LCH)],
                            start=True, stop=True)
                    pt = pt_tile()
                    nc.scalar.activation(
                        pt, sp, func=mybir.ActivationFunctionType.Exp,
                        scale=1.0 / np.sqrt(DHEAD))
                    pt_tiles[h][lt] = pt
                    fi = pump(fl, fi, lt)
        return pt_tiles

    # phase A: (p0, e0); fillers: p1's K/Q-ec0 first (phase B needs them),
    # then V^T proj + V DMA-transposes for both pairs, Q ec1 for p0
    fillA = [lambda: qkv_chunk(kT[0], 0, 1, 1),
             lambda: qkv_chunk(kT[0], 0, 1, 2),
             lambda: qkv_chunk(kT[0], 0, 1, 3),
             lambda: qkv_chunk(kT[1], 1, 1, 0),
             lambda: qkv_chunk(kT[1], 1, 1, 1),
             lambda: qkv_chunk(kT[1], 1, 1, 2),
             lambda: qkv_chunk(kT[1], 1, 1, 3),
             lambda: qkv_chunk(qT[1], 1, 0, 0),
             lambda: qkv_chunk(qT[1], 1, 0, 1),
             ones_unit(0), ones_unit(1)]
    for p in range(NPAIR):
        for g in range(4):
            fillA.append(vT_chunk_unit(p, g))
            fillA.append(v_trans_unit(p, range(4 * g, 4 * g + 4)))
    fillA += [q1_unit(0, NSUB + 0), q1_unit(0, NSUB + 1)]
    ptA = phase(0, 0, fillA)

    # phase B: (p1, e0); fillers: attend(p0,e0) h-major (frees pt(A) fast)
    fillB = [attend_unit(ptA, 0, 0, 0, 0),
             attend_unit(ptA, 0, 0, 1, 0),
             attend_unit(ptA, 0, 0, 0, 1),
             attend_unit(ptA, 0, 0, 1, 1)]
    ptB = phase(1, 0, fillB)

    # phase C: (p0, e1); fillers: attend(p1,e0), Q ec1 for p1, then
    # out-proj rows 0:512
    fillC = [attend_unit(ptB, 1, 0, 0, 0),
             attend_unit(ptB, 1, 0, 1, 0),
             attend_unit(ptB, 1, 0, 0, 1),
             attend_unit(ptB, 1, 0, 1, 1),
             q1_unit(1, NSUB + 0), q1_unit(1, NSUB + 1)]
    fillC += [outproj_unit(lt) for lt in range(4)]
    ptC = phase(0, 1, fillC)

    # phase D: (p1, e1); fillers: attend(p0,e1) + out-proj rows 512:1024
    fillD = [attend_unit(ptC, 0, 1, 0, 0),
             attend_unit(ptC, 0, 1, 1, 0),
             attend_unit(ptC, 0, 1, 0, 1),
             attend_unit(ptC, 0, 1, 1, 1)]
    fillD += [outproj_unit(lt) for lt in range(4, 8)]
    ptD = phase(1, 1, fillD)

    # tail: attend(p1,e1) + out-proj rows 1024:2048, interleaved by
    # o_norm readiness
    attend_unit(ptD, 1, 1, 0, 0)()
    attend_unit(ptD, 1, 1, 0, 1)()
    attend_unit(ptD, 1, 1, 1, 0)()
    for lt in range(8, 12):
        outproj_unit(lt, tail=True)()
    attend_unit(ptD, 1, 1, 1, 1)()
    for lt in range(12, 16):
        outproj_unit(lt, tail=True)()

    if dbg:
        nc.sync.dma_start(dbg["kT0"], kT[0])
        nc.sync.dma_start(dbg["qT0"], qT[0])
        nc.sync.dma_start(dbg["vT0"], vTs[0])
        nc.sync.dma_start(dbg["vt0"], vt[0])
        nc.sync.dma_start(dbg["on0"], o_norm)
        nc.sync.dma_start(dbg["pt0"], ptD[0][0])
        nc.sync.dma_start(dbg["pt15"], ptD[1][15])


_NC_CACHE = {}


def _get_nc():
    if "nc" not in _NC_CACHE:
        _NC_CACHE["nc"] = build_nc()
    return _NC_CACHE["nc"]


def shard_inputs(x, Wq, bq, Wk, bk, Wv, bv, Wd, bd):
    """Build the 8 per-core input maps (layout marshalling only)."""
    in_maps = []
    x = np.asarray(x, np.float32)
    for c in range(NCORES):
        b = c // (NCORES // B)
        h0 = (c % (NCORES // B)) * H_PER_CORE
        hs = slice(h0, h0 + H_PER_CORE)
        in_maps.append({
            "xt": np.ascontiguousarray(x[b].T).astype(BF16_NP),
            "wq": np.ascontiguousarray(np.asarray(Wq[:, hs, :]).reshape(DMODEL, -1)).astype(BF16_NP),
            "wk": np.ascontiguousarray(np.asarray(Wk[:, hs, :]).reshape(DMODEL, -1)).astype(BF16_NP),
            "wv": np.ascontiguousarray(np.asarray(Wv[:, hs, :]).reshape(DMODEL, -1)).astype(BF16_NP),
            "wd": np.ascontiguousarray(np.asarray(Wd[hs]).reshape(-1, DMODEL)).astype(BF16_NP),
            "bq": np.ascontiguousarray(np.asarray(bq[hs], np.float32).reshape(-1)),
            "bk": np.ascontiguousarray(np.asarray(bk[hs], np.float32).reshape(-1)),
            "bv": np.ascontiguousarray(np.asarray(bv[hs], np.float32).reshape(-1)),
        })
    return in_maps


def gather_outputs(results, bd):
    """Sum partial outputs per batch and add bd."""
    out = np.zeros((B, L, DMODEL), np.float32)
    per_b = NCORES // B
    for c, res in enumerate(results):
        out[c // per_b] += res["y"]
    out += np.asarray(bd, np.float32)[None, None, :]
    return out


def kernel(x, Wq, bq, Wk, bk, Wv, bv, Wd, bd, _trace=False):
    nc = _get_nc()
    in_maps = shard_inputs(x, Wq, bq, Wk, bk, Wv, bv, Wd, bd)
    res = run_bass_kernel_spmd(nc, in_maps, list(range(NCORES)), trace=_trace)
    out = gather_outputs(res.results, bd)
    if _trace:
        kernel.last_results = res
    return out


# revision 43
# speedup vs baseline: 1.1990x; 1.1797x over previous
"""Trainium2 Bass kernel: multi-head attention (dense transformer block).

Computation (per batch b):
    Q = x @ Wq + bq ; K = x @ Wk + bk ; V = x @ Wv + bv        (per head)
    P = exp((Q @ K^T) / sqrt(Dh))            (no max-subtraction: scores O(1))
    out = sum_h (P @ V / rowsum(P)) @ Wd[h] + bd

Sharding (data + tensor parallel): 8 cores; core c handles batch b = c // 4
and the 4 heads starting at 4*(c % 4). Each core computes a partial [L, D]
output; the host sums the 4 partials per batch and adds bd. Host-side input
marshalling (layout only, no FLOPs): x is passed pre-transposed as bf16
x^T [DMODEL, L] per core and the weight slices as bf16, so the device
spends no time on x transposes or fp32->bf16 weight casts.

Schedule: the span is bounded by PE matmul streaming (~175us) with the
Scalar(ACT) exp stream (~125us) hidden under it:
  - 4 "phases", one per (pair, exp-chunk): scores + exp, paced by PSUM
    recycling. Between score l-tiles we pump "filler" PE work that is
    already data-ready: V^T projection (phase A), the previous phase's
    attend chunks, Q ec1 projection, out-projection + y DMA (phases B-D).
  - scores run as two concurrent 64-row PE tiles (tile_position row groups
    0/64), interleaved h0/h1 so the second tile's LDWEIGHTS pulls ahead.
  - V is produced as V^T (weights stationary, N=512 streams) and moved into
    [l', d] attend layout by DMA-transpose (2-byte xbar path) - zero PE cost.
  - softmax denominator via 64 replicated ones-columns in the attend
    stationary operand (free: matmul time ~ N only).
  - normalize: DVE copy (frees PSUM), DVE reciprocal_approx_fast, multiply
    on the otherwise-idle GpSimd.
  - ACT does exp ONLY while busy (weight DMAs are issued from its idle DGE
    queue at startup; tail y drains use it after the last exp).
All matmuls bf16 (fp32 PSUM accumulation); rel err vs fp32 ref ~5e-3.
"""

import os
import sys
from contextlib import ExitStack

import numpy as np
import ml_dtypes

for _p in ("/opt/trn_rl_repo", "/root/.axon_site/_ro/trn_rl_repo"):
    if os.path.isdir(_p) and _p not in sys.path:
        sys.path.append(_p)

import concourse.bass as bass
import concourse.tile as tile
from concourse import bacc, mybir
from concourse.bass import ds
from concourse.bass_utils import run_bass_kernel_spmd

F32 = mybir.dt.float32
BF16 = mybir.dt.bfloat16
BF16_NP = ml_dtypes.bfloat16

# Problem sizes (hardcoded per contract).
DMODEL, HEADS, DHEAD = 1024, 16, 64
B, L = 2, 2048
NCORES = 8
H_PER_CORE = B * HEADS // NCORES          # 4 heads per core
NPAIR = H_PER_CORE // 2                   # head pairs per core
P = 128                                   # partitions
KT = DMODEL // P                          # 8 k-tiles over dmodel
NLT = L // P                              # 16 l-tiles
LCH = 512                                 # matmul free-dim chunk (one psum bank)
ECH = 1024                                # exp chunk (2 psum banks)
NEC = L // ECH                            # 2 exp chunks
MCH = 512                                 # m-chunk for out-proj
NMC = DMODEL // MCH
NSUB = ECH // LCH                         # 2 sub-chunks per exp chunk


def build_nc():
    """Build the SPMD Bass program for one core."""
    nc = bacc.Bacc("TRN2", target_bir_lowering=False, debug=False,
                   num_devices=NCORES)

    xt_d = nc.dram_tensor("xt", [DMODEL, L], BF16, kind="ExternalInput").ap()
    wq_d = nc.dram_tensor("wq", [DMODEL, H_PER_CORE * DHEAD], BF16, kind="ExternalInput").ap()
    wk_d = nc.dram_tensor("wk", [DMODEL, H_PER_CORE * DHEAD], BF16, kind="ExternalInput").ap()
    wv_d = nc.dram_tensor("wv", [DMODEL, H_PER_CORE * DHEAD], BF16, kind="ExternalInput").ap()
    wd_d = nc.dram_tensor("wd", [H_PER_CORE * DHEAD, DMODEL], BF16, kind="ExternalInput").ap()
    bq_d = nc.dram_tensor("bq", [H_PER_CORE * DHEAD], F32, kind="ExternalInput").ap()
    bk_d = nc.dram_tensor("bk", [H_PER_CORE * DHEAD], F32, kind="ExternalInput").ap()
    bv_d = nc.dram_tensor("bv", [H_PER_CORE * DHEAD], F32, kind="ExternalInput").ap()
    y_d = nc.dram_tensor("y", [L, DMODEL], F32, kind="ExternalOutput").ap()
    dbg = {}
    if os.environ.get("K_DEBUG"):
        for nm, shape in (("kT0", [P, L]), ("qT0", [P, L]), ("vT0", [P, L]),
                          ("vt0", [P, NLT, 2, P]), ("on0", [P, NPAIR, L]),
                          ("pt0", [P, ECH]), ("pt15", [P, ECH])):
            dbg[nm] = nc.dram_tensor(nm, shape, BF16, kind="ExternalOutput").ap()

    with ExitStack() as ctx:
        tc = ctx.enter_context(tile.TileContext(nc))
        _body(nc, tc, ctx, xt_d, wq_d, wk_d, wv_d, wd_d, bq_d, bk_d, bv_d, y_d,
              dbg)
    nc.compile()
    return nc


def _body(nc, tc, ctx, xt_d, wq_d, wk_d, wv_d, wd_d, bq_d, bk_d, bv_d, y_d,
          dbg=None):
    const = ctx.enter_context(tc.tile_pool(name="const", bufs=1))
    sb = ctx.enter_context(tc.tile_pool(name="sb", bufs=1))
    psum = ctx.enter_context(tc.tile_pool(name="psum", bufs=1, space="PSUM"))

    # biases via gpsimd SWDGE (off the hw queues)
    bias_sb = const.tile([P, 3, NPAIR], F32)
    for i, b_d in enumerate((bq_d, bk_d, bv_d)):
        for p in range(NPAIR):
            nc.gpsimd.dma_start(bias_sb[:, i, p:p + 1],
                                b_d.rearrange("(a p) -> a p", p=P)[p:p + 1, :]
                                .rearrange("a p -> p a"))

    # ---- weights: bf16 from host, DMA'd on the scalar queue (idle now);
    # ordered so the K projections (emitted first) unblock earliest ----
    w_sb = const.tile([P, NPAIR, 3, KT, P], BF16)
    wd_sb = const.tile([P, NPAIR, DMODEL], BF16)
    for i, p in ((1, 0), (0, 0), (1, 1), (0, 1), (2, 0), (2, 1)):
        w_d = (wq_d, wk_d, wv_d)[i]
        nc.scalar.dma_start(
            w_sb[:, p, i],
            w_d.rearrange("(kt k) m -> k kt m", k=P)[:, :, ds(p * P, P)])
    nc.scalar.dma_start(wd_sb, wd_d.rearrange("(pp k) m -> k pp m", k=P))

    # ---- x^T: bf16 from host, 4 L-chunk DMAs so K-proj starts early ----
    xt = sb.tile([P, KT, L], BF16)
    for lc in range(4):
        for kh in range(2):
            nc.sync.dma_start(
                xt[:, ds(4 * kh, 4), ds(lc * LCH, LCH)],
                xt_d.rearrange("(kt p) l -> p kt l", p=P)
                [:, ds(4 * kh, 4), ds(lc * LCH, LCH)])

    # ---- persistent activations ----
    kT = [sb.tile([P, L], BF16, tag=f"kT{p}", bufs=1, name=f"kT{p}")
          for p in range(NPAIR)]
    qT = [sb.tile([P, L], BF16, tag=f"qT{p}", bufs=1, name=f"qT{p}")
          for p in range(NPAIR)]
    vTs = [sb.tile([P, L], BF16, tag=f"vT{p}", bufs=1, name=f"vT{p}")
           for p in range(NPAIR)]
    vt = [sb.tile([P, NLT, 2 * P], BF16, tag=f"vt{p}", bufs=1, name=f"vt{p}")
          for p in range(NPAIR)]
    o_norm = sb.tile([P, NPAIR, L], BF16)

    # pt tiles: exp outputs; ring holds the previous phase's 32 tiles plus
    # the current phase's in-flight ones.
    PT_BUFS = 42

    def pt_tile():
        return sb.tile([P, ECH], BF16, tag="pt", bufs=PT_BUFS, name="pt")

    # psum: scores 2x[128,1024] (4 banks) + attend op 2x[128,512] (2 banks)
    # + aux (projections/out-proj) 2x[128,512] (2 banks)
    def sc_tile():
        return psum.tile([P, ECH], F32, tag="sctr", bufs=2, name="sctr")

    def aux_tile(shape, dt=F32, name="aux"):
        return psum.tile(shape, dt, tag="aux", bufs=2, name=name)

    # ---- helper: one qkv projection 512-chunk (8 accumulating matmuls) ----
    def qkv_chunk(dst, p, i, lc):
        ps = aux_tile([P, LCH], name="qkvp")
        for kt in range(KT):
            nc.tensor.matmul(
                ps, lhsT=w_sb[:, p, i, kt],
                rhs=xt[:, kt, ds(lc * LCH, LCH)],
                start=(kt == 0), stop=(kt == KT - 1))
        nc.vector.tensor_scalar_add(
            dst[:, ds(lc * LCH, LCH)], ps, bias_sb[:, i, p:p + 1])

    # ---- filler units ----
    def vT_chunk_unit(p, lc):
        def emit():
            qkv_chunk(vTs[p], p, 2, lc)
        return emit

    def v_trans_unit(p, lts):
        """One DMA-transpose per l-tile into vt layout
        [1s(0:64) | V_h0(64:128) | V_h1(128:192) | 1s(192:256)], so
        h0's attend lhsT = cols 0:128 = [ones | V] and h1's = 128:256 =
        [V | ones] - both contiguous."""
        def emit():
            for lt in lts:
                nc.sync.dma_start_transpose(
                    vt[p][:, lt, ds(DHEAD, 2 * DHEAD)],
                    vTs[p][:, ds(lt * P, P)])
        return emit

    def ones_unit(p):
        def emit():
            nc.vector.memset(vt[p][:, :, 0:DHEAD], 1.0)
            nc.vector.memset(vt[p][:, :, 3 * DHEAD:4 * DHEAD], 1.0)
        return emit

    def q1_unit(p, lc):
        def emit():
            qkv_chunk(qT[p], p, 0, lc)
        return emit

    def attend_unit(pt_tiles, p, ec, sub, h):
        """One attend chunk: 16 accumulating matmuls + normalize chain."""
        def emit():
            lc = ec * ECH + sub * LCH
            op = psum.tile([P, LCH], F32, tag="op", bufs=2, name="op")
            for lt in range(NLT):
                nc.tensor.matmul(
                    op, lhsT=vt[p][:, lt, ds(P * h, P)],
                    rhs=pt_tiles[h][lt][:, ds(sub * LCH, LCH)],
                    start=(lt == 0), stop=(lt == NLT - 1))
            # h0: rows 0:64 = denominator, 64:128 = O; h1: the reverse.
            # Both halves are copied to base-partition-0 tiles (custom DVE
            # ops and TensorTensor need base-0-aligned SBUF operands)
            o_rows = ds(DHEAD, DHEAD) if h == 0 else ds(0, DHEAD)
            d_rows = ds(0, DHEAD) if h == 0 else ds(DHEAD, DHEAD)
            os_sb = sb.tile([DHEAD, LCH], F32, tag="os", bufs=2, name="os_sb")
            nc.vector.tensor_copy(os_sb, op[o_rows, :])
            den = sb.tile([DHEAD, LCH], F32, tag="den", bufs=2, name="den")
            nc.vector.tensor_copy(den, op[d_rows, :])
            rs = sb.tile([DHEAD, LCH], F32, tag="rs", bufs=2, name="rs")
            nc.vector.reciprocal_approx_fast(rs, den)
            nc.gpsimd.tensor_mul(
                o_norm[ds(64 * h, 64), p, ds(lc, LCH)],
                os_sb, rs)
        return emit

    def outproj_unit(lt, tail=False):
        """Y[lt-tile, :] = sum_pairs o_norm^T @ Wd, drain + DMA out."""
        def emit():
            for mc in range(NMC):
                yp = aux_tile([P, MCH], name="yp")
                for p in range(NPAIR):
                    nc.tensor.matmul(
                        yp, lhsT=o_norm[:, p, ds(lt * P, P)],
                        rhs=wd_sb[:, p, ds(mc * MCH, MCH)],
                        start=(p == 0), stop=(p == NPAIR - 1))
                ys = sb.tile([P, MCH], F32, tag="ys", bufs=3, name="ys")
                if tail and mc % 2 == 1:
                    nc.scalar.copy(ys, yp)   # ACT is idle after the last exp
                else:
                    nc.vector.tensor_copy(ys, yp)
                nc.sync.dma_start(y_d[ds(lt * P, P), ds(mc * MCH, MCH)], ys)
        return emit

    # ---- startup: the minimum for phase A's first score tiles (K g0 +
    # Q ec0); everything else becomes phase-A fillers so the first exp
    # fires as early as the DMAs allow ----
    qkv_chunk(kT[0], 0, 1, 0)
    for lc in range(NSUB):
        qkv_chunk(qT[0], 0, 0, lc)

    # ---- phases: scores+exp paced by ACT; fillers keep PE dense ----
    def phase(p, ec, fillers, h_major=False, fillers2=None, out_pt=None):
        """Scores+exp for (pair p, exp chunk ec); pump filler units between
        l-tiles, front-loaded (done ~3 l-tiles early) so the PE reaches the
        next phase's scores before this phase's last exps finish. With
        h_major, the two heads run as separate sweeps (fillers2 pumps during
        the h1 sweep - it may reference this phase's own h0 pt tiles).
        Returns the pt tiles for this phase."""
        pt_tiles = out_pt if out_pt is not None else [[None] * NLT, [None] * NLT]

        def pump(fillers, fi, lt):
            nf = len(fillers)
            while fi < nf and fi * NLT < nf * (lt + 1):
                fillers[fi]()
                fi += 1
            return fi

        if not h_major:
            fi = 0
            for lt in range(NLT):
                sp = [sc_tile(), sc_tile()]
                for sub in range(NSUB):   # h-interleaved: dual 64-row tiles
                    for h in range(2):
                        nc.tensor.matmul(
                            sp[h][:, ds(sub * LCH, LCH)],
                            lhsT=kT[p][ds(64 * h, 64), ds(lt * P, P)],
                            rhs=qT[p][ds(64 * h, 64),
                                      ds(ec * ECH + sub * LCH, LCH)],
                            start=True, stop=True)
                for h in range(2):
                    pt = pt_tile()
                    nc.scalar.activation(
                        pt, sp[h], func=mybir.ActivationFunctionType.Exp,
                        scale=1.0 / np.sqrt(DHEAD))
                    pt_tiles[h][lt] = pt
                fi = pump(fillers, fi, lt)
        else:
            for h in range(2):
                fl = fillers if h == 0 else (fillers2 or [])
                fi = 0
                for lt in range(NLT):
                    sp = sc_tile()
                    for sub in range(NSUB):
                        nc.tensor.matmul(
                            sp[:, ds(sub * LCH, LCH)],
                            lhsT=kT[p][ds(64 * h, 64), ds(lt * P, P)],
                            rhs=qT[p][ds(64 * h, 64),
                                      ds(ec * ECH + sub * LCH, LCH)],
                            start=True, stop=True)
                    pt = pt_tile()
                    nc.scalar.activation(
                        pt, sp, func=mybir.ActivationFunctionType.Exp,
                        scale=1.0 / np.sqrt(DHEAD))
                    pt_tiles[h][lt] = pt
                    fi = pump(fl, fi, lt)
        return pt_tiles

    # phase A: (p0, e0); fillers: p1's K/Q-ec0 first (phase B needs them),
    # then V^T proj + V DMA-transposes for both pairs, Q ec1 for p0
    fillA = [lambda: qkv_chunk(kT[0], 0, 1, 1),
             lambda: qkv_chunk(kT[0], 0, 1, 2),
             lambda: qkv_chunk(kT[0], 0, 1, 3),
             lambda: qkv_chunk(kT[1], 1, 1, 0),
             lambda: qkv_chunk(kT[1], 1, 1, 1),
             lambda: qkv_chunk(kT[1], 1, 1, 2),
             lambda: qkv_chunk(kT[1], 1, 1, 3),
             lambda: qkv_chunk(qT[1], 1, 0, 0),
             lambda: qkv_chunk(qT[1], 1, 0, 1),
             ones_unit(0), ones_unit(1)]
    for p in range(NPAIR):
        for g in range(4):
            fillA.append(vT_chunk_unit(p, g))
            fillA.append(v_trans_unit(p, range(4 * g, 4 * g + 4)))
    fillA += [q1_unit(0, NSUB + 0)]
    ptA = phase(0, 0, fillA)

    # phase B: (p1, e0); fillers: attend(p0,e0) h-major (frees pt(A) fast)
    fillB = [q1_unit(0, NSUB + 1),
             attend_unit(ptA, 0, 0, 0, 0),
             attend_unit(ptA, 0, 0, 1, 0),
             attend_unit(ptA, 0, 0, 0, 1)]
    ptB = phase(1, 0, fillB)

    # phase C: (p0, e1); fillers: attend(p1,e0), Q ec1 for p1, then
    # out-proj rows 0:512
    fillC = [attend_unit(ptA, 0, 0, 1, 1),
             attend_unit(ptB, 1, 0, 0, 0),
             attend_unit(ptB, 1, 0, 1, 0),
             attend_unit(ptB, 1, 0, 0, 1),
             attend_unit(ptB, 1, 0, 1, 1),
             q1_unit(1, NSUB + 0), q1_unit(1, NSUB + 1)]
    fillC += [outproj_unit(lt) for lt in range(3)]
    ptC = phase(0, 1, fillC)

    # phase D: (p1, e1); fillers: attend(p0,e1) + out-proj rows 512:1024
    fillD = [outproj_unit(3),
             attend_unit(ptC, 0, 1, 0, 0),
             attend_unit(ptC, 0, 1, 1, 0),
             attend_unit(ptC, 0, 1, 0, 1),
             attend_unit(ptC, 0, 1, 1, 1)]
    fillD += [outproj_unit(lt) for lt in range(4, 7)]
    ptD = phase(1, 1, fillD)

    # tail: attend(p1,e1) + out-proj rows 1024:2048, interleaved by
    # o_norm readiness
    outproj_unit(7)()
    attend_unit(ptD, 1, 1, 0, 0)()
    attend_unit(ptD, 1, 1, 0, 1)()
    attend_unit(ptD, 1, 1, 1, 0)()
    for lt in range(8, 12):
        outproj_unit(lt, tail=True)()
    attend_unit(ptD, 1, 1, 1, 1)()
    for lt in range(12, 16):
        outproj_unit(lt, tail=True)()

    if dbg:
        nc.sync.dma_start(dbg["kT0"], kT[0])
        nc.sync.dma_start(dbg["qT0"], qT[0])
        nc.sync.dma_start(dbg["vT0"], vTs[0])
        nc.sync.dma_start(dbg["vt0"], vt[0])
        nc.sync.dma_start(dbg["on0"], o_norm)
        nc.sync.dma_start(dbg["pt0"], ptD[0][0])
        nc.sync.dma_start(dbg["pt15"], ptD[1][15])


_NC_CACHE = {}


def _get_nc():
    if "nc" not in _NC_CACHE:
        _NC_CACHE["nc"] = build_nc()
    return _NC_CACHE["nc"]


def shard_inputs(x, Wq, bq, Wk, bk, Wv, bv, Wd, bd):
    """Build the 8 per-core input maps (layout marshalling only)."""
    in_maps = []
    x = np.asarray(x, np.float32)
    for c in range(NCORES):
        b = c // (NCORES // B)
        h0 = (c % (NCORES // B)) * H_PER_CORE
        hs = slice(h0, h0 + H_PER_CORE)
        in_maps.append({
            "xt": np.ascontiguousarray(x[b].T).astype(BF16_NP),
            "wq": np.ascontiguousarray(np.asarray(Wq[:, hs, :]).reshape(DMODEL, -1)).astype(BF16_NP),
            "wk": np.ascontiguousarray(np.asarray(Wk[:, hs, :]).reshape(DMODEL, -1)).astype(BF16_NP),
            "wv": np.ascontiguousarray(np.asarray(Wv[:, hs, :]).reshape(DMODEL, -1)).astype(BF16_NP),
            "wd": np.ascontiguousarray(np.asarray(Wd[hs]).reshape(-1, DMODEL)).astype(BF16_NP),
            "bq": np.ascontiguousarray(np.asarray(bq[hs], np.float32).reshape(-1)),
            "bk": np.ascontiguousarray(np.asarray(bk[hs], np.float32).reshape(-1)),
            "bv": np.ascontiguousarray(np.asarray(bv[hs], np.float32).reshape(-1)),
        })
    return in_maps


def gather_outputs(results, bd):
    """Sum partial outputs per batch and add bd."""
    out = np.zeros((B, L, DMODEL), np.float32)
    per_b = NCORES // B
    for c, res in enumerate(results):
        out[c // per_b] += res["y"]
    out += np.asarray(bd, np.float32)[None, None, :]
    return out


def kernel(x, Wq, bq, Wk, bk, Wv, bv, Wd, bd, _trace=False):
    nc = _get_nc()
    in_maps = shard_inputs(x, Wq, bq, Wk, bk, Wv, bv, Wd, bd)
    res = run_bass_kernel_spmd(nc, in_maps, list(range(NCORES)), trace=_trace)
    out = gather_outputs(res.results, bd)
    if _trace:
        kernel.last_results = res
    return out
